# revision 24
# baseline (speedup 1.0000x reference)
"""Trainium2 Bass kernel for nn_CRF_SelfAttention_65627100283470.

Math (validated vs the reference at 1e-6 rel err):
  - The CRF/marginal branch is dead code: softmax over the class dim sums
    to 1, so sum(cluster_features, 0) == sum of context rows.  The output
    is (sum_{f,p} context2) @ cls_W + cls_b.
  - context2 = w2*T2 + w1*(1-w2)*T1 with T_it the per-iteration temporal
    tensors, and w_it per-frame halting weights -> only per-frame sums of
    temporal are needed at the end.
  - QKV projections are shared across overlapping windows; exp(scores)
    strips are shared across windows (computed per key-frame strip); the
    output projection commutes with overlap-add; softmax denominators come
    from a ones-column prepended to V.

Sharding: 8 heads -> 8 cores.  One AllReduce of the temporal between the
two iterations + one tiny final AllReduce.

This revision (vs the 507us baseline) restructures for PE-instruction
economy:
  - Q/K/V projections grouped across the 3 scales ([96|96|97] row blocks)
  - A@V windows write partition-offset slices of shared [99, s*128] PSUM
    group tiles (3 windows per tile) -> 3x fewer drains + transposes;
    j-major sweep dedups LDWEIGHTS.
  - V' build and abar->abarT transposes moved to DMA-engine transposes.
  - Wo contracts all 3 scales at once (K=96) -> 10 matmuls per iter.
  - iter 2 skips the full Wo projection (only per-frame sums needed).
  - halting mean via [18,128] layout (1 transpose instead of 18).
"""
import sys
import types

import numpy as np

F, P, H, HEADS, C, NCLS = 18, 128, 256, 8, 32, 625
SCALES = (2, 4, 6)
HD = H // HEADS
NTOK = F * P  # 2304
NCORES = 8

# A@V matmuls write at partition offset 33*wi of the shared group tile.
# If partition-offset PSUM writes are not honored, flip to False to use
# zero-padded 99-row weight slices instead.
_OFFSET_WRITE = False
_DEBUG_DUMPS = False

# frame block pitch inside vp (ones+V layout with zero guards)
_VPITCH = 352  # 16-aligned V blocks: ones at 79+112*si, V at 80+112*si


def _enable_ldw_opt():
    """Walrus's LDWEIGHTS dedup is disabled by default in bass_utils;
    enable it (verified numerically by the rel-err gate in test.py)."""
    import concourse.bass_utils as bu

    if getattr(bu, "_ldw_opt_patched", False):
        return
    orig = bu.bir_verify_and_optimise

    def patched(*args, **kwargs):
        real_run = bu.run_command

        def run_hook(argv, **kw):
            argv = ["--enable-ldw-opt=true" if a == "--enable-ldw-opt=false"
                    else a for a in argv]
            return real_run(argv, **kw)

        bu.run_command = run_hook
        try:
            return orig(*args, **kwargs)
        finally:
            bu.run_command = real_run

    bu.bir_verify_and_optimise = patched
    bu._ldw_opt_patched = True


def _install_ntff_hook():
    """Recreate the missing antenv.axon_hooks so trace=True works."""
    if "antenv.axon_hooks" in sys.modules:
        return
    try:
        import antenv

        mod = types.ModuleType("antenv.axon_hooks")
        mod._hook = None
        mod.set_axon_ntff_profile_hook = lambda h: setattr(mod, "_hook", h)
        mod.get_axon_ntff_profile_hook = lambda: mod._hook
        sys.modules["antenv.axon_hooks"] = mod
        antenv.axon_hooks = mod
        from trn_agent_boot.trn_boot import _ntff_profile_via_ctypes

        mod.set_axon_ntff_profile_hook(
            _ntff_profile_via_ctypes("/opt/axon/libaxon_pjrt.so")
        )
    except Exception:
        pass


def _chunks(n, lim=512):
    out = [lim] * (n // lim)
    if n % lim:
        out.append(n % lim)
    return out


def _counts(s):
    nw = F - s + 1
    c = np.zeros(F, np.float32)
    for w in range(nw):
        c[w:w + s] += 1.0
    return c


def _strip_meta(s):
    """Per key-frame strip [a, b] ranges and col offsets in the est tile."""
    offs, rng = [], []
    off = 0
    for f2 in range(F):
        a = max(0, f2 - s + 1)
        b = min(F - 1, f2 + s - 1)
        offs.append(off)
        rng.append((a, b))
        off += (b - a + 1) * 128
    return offs, rng, off


def build():
    import concourse.bacc as bacc
    import concourse.mybir as mybir
    from concourse.tile import TileContext

    dt = mybir.dt
    f32 = dt.float32
    bf16 = dt.bfloat16
    AF = mybir.ActivationFunctionType
    ALU = mybir.AluOpType

    nc = bacc.Bacc("TRN2", target_bir_lowering=False, debug=False,
                   num_devices=NCORES)

    # ---- I/O ----
    xt_in = nc.dram_tensor("xt", [2, 128, NTOK], bf16, kind="ExternalInput")
    wq_in = nc.dram_tensor("wq", [2, 128, 96], bf16, kind="ExternalInput")
    wk_in = nc.dram_tensor("wk", [2, 128, 96], bf16, kind="ExternalInput")
    wv_in = nc.dram_tensor("wv", [2, 128, 97], bf16, kind="ExternalInput")
    wo_in = nc.dram_tensor("wo", [96, 256], bf16, kind="ExternalInput")
    boq_in = nc.dram_tensor("boq", [2, 128, 1], f32, kind="ExternalInput")
    nhb_in = nc.dram_tensor("nhb", [18, 1], f32, kind="ExternalInput")
    cinv_in = nc.dram_tensor("cinv", [3, 128, F], f32, kind="ExternalInput")
    clsw_in = nc.dram_tensor("clsw", [2, 128, NCLS], f32, kind="ExternalInput")
    clsb_in = nc.dram_tensor("clsb", [1, NCLS], f32, kind="ExternalInput")
    id_in = nc.dram_tensor("ident", [128, 128], f32, kind="ExternalInput")
    out_d = nc.dram_tensor("out", [1, NCLS], f32, kind="ExternalOutput")

    HALF = NTOK // 2  # 1152
    ar_in = [nc.dram_tensor(f"ar_in{h}", [2, 128, HALF], bf16)
             for h in range(2)]
    ar_out = [nc.dram_tensor(f"ar_out{h}", [2, 128, HALF], bf16,
                             addr_space="Shared") for h in range(2)]
    ar2_in = nc.dram_tensor("ar2_in", [2, 128, 1], f32)
    hbounce = nc.dram_tensor("hbounce", [18, 128], bf16)
    dbg = {
        "QT": nc.dram_tensor("dbg_QT", [96, NTOK], bf16, kind="ExternalOutput"),
        "VT": nc.dram_tensor("dbg_VT", [97, NTOK], bf16, kind="ExternalOutput"),
        "vp": nc.dram_tensor("dbg_vp", [128, F * _VPITCH + 64], bf16,
                             kind="ExternalOutput"),
        "est6": nc.dram_tensor("dbg_est6", [128, 21504], bf16,
                               kind="ExternalOutput"),
        "abar": nc.dram_tensor("dbg_abar", [128, F * 128], mybir.dt.float32,
                               kind="ExternalOutput"),
        "abarT": nc.dram_tensor("dbg_abarT", [128, NTOK], bf16,
                                kind="ExternalOutput"),
        "xt0": nc.dram_tensor("dbg_xt0", [128, NTOK], bf16,
                              kind="ExternalOutput"),
        "wt0": nc.dram_tensor("dbg_wt0", [1, F], mybir.dt.float32,
                              kind="ExternalOutput"),
        "ssum00": nc.dram_tensor("dbg_ssum00", [128, F], mybir.dt.float32,
                                 kind="ExternalOutput"),
    }
    ar2_out = nc.dram_tensor("ar2_out", [2, 128, 1], f32, addr_space="Shared")

    col_cc = _chunks(NTOK)  # [512]*4 + [256]
    half_cc = _chunks(NTOK // 2)  # [512, 512, 128]
    meta = {s: _strip_meta(s) for s in SCALES}

    with TileContext(nc) as tc:
        with (
            tc.tile_pool(name="pin", bufs=1) as pin,
            tc.tile_pool(name="work", bufs=3) as work,
        ):
            # ---- persistent tiles + weight loads ----
            xt = [[pin.tile([128, HALF], bf16, tag=f"xt{c}{h}",
                            name=f"xt{c}{h}") for h in range(2)]
                  for c in range(2)]
            wq_t = pin.tile([128, 2 * 96], bf16, tag="wq")
            wk_t = pin.tile([128, 2 * 96], bf16, tag="wk")
            wv_t = pin.tile([128, 2 * 97], bf16, tag="wv")
            wo_t = pin.tile([96, 256], bf16, tag="wo")
            boq = pin.tile([128, 2], f32, tag="boq")
            boq128 = pin.tile([128, 2], f32, tag="boq128")
            nhb = pin.tile([18, 1], f32, tag="nhb")
            cinv = pin.tile([128, 3 * F], f32, tag="cinv")
            clsw = pin.tile([128, 2 * NCLS], f32, tag="clsw")
            clsb = pin.tile([1, NCLS], f32, tag="clsb")
            ident = pin.tile([128, 128], f32, tag="ident")
            identb = pin.tile([128, 128], bf16, tag="identb")
            ones_row = pin.tile([1, 128], f32, tag="ones_row")
            ones_col = pin.tile([128, 1], f32, tag="ones_col")

            for c in range(2):
                for h in range(2):
                    nc.sync.dma_start(out=xt[c][h][:],
                                      in_=xt_in[c, :, h * HALF:(h + 1) * HALF])
                nc.sync.dma_start(out=wq_t[:, c * 96:(c + 1) * 96], in_=wq_in[c])
                nc.sync.dma_start(out=wk_t[:, c * 96:(c + 1) * 96], in_=wk_in[c])
                nc.sync.dma_start(out=wv_t[:, c * 97:(c + 1) * 97], in_=wv_in[c])
                nc.gpsimd.dma_start(out=boq[:, c:c + 1], in_=boq_in[c])
                nc.gpsimd.dma_start(out=clsw[:, c * NCLS:(c + 1) * NCLS],
                                    in_=clsw_in[c])
            nc.gpsimd.dma_start(out=wo_t[:], in_=wo_in[:])
            nc.sync.dma_start(out=nhb[:], in_=nhb_in[:])
            for si in range(3):
                nc.gpsimd.dma_start(out=cinv[:, si * F:(si + 1) * F],
                                    in_=cinv_in[si])
            nc.gpsimd.dma_start(out=clsb[:], in_=clsb_in[:])
            nc.gpsimd.dma_start(out=ident[:], in_=id_in[:])
            nc.vector.memset(ones_row[:], 1.0)
            nc.vector.memset(ones_col[:], 1.0)
            nc.vector.tensor_copy(identb[:], ident[:])
            nc.vector.tensor_scalar_mul(out=boq128[:], in0=boq[:],
                                        scalar1=128.0)

            # grouped projections (token cols)
            QT = pin.tile([96, NTOK], bf16, tag="QT")
            KT = pin.tile([96, NTOK], bf16, tag="KT")
            VT = pin.tile([97, NTOK], bf16, tag="VT")
            # V' tile: per frame [z66 |1|V2| z33.., ones at 66+99*si]
            vp = pin.tile([128, F * _VPITCH + 64], bf16, tag="vp")
            vstage = pin.tile([128, F * 96], bf16, tag="vstage")
            nc.vector.memset(vp[:], 0.0)
            for si in range(3):
                nc.vector.memset(
                    vp[:, :F * _VPITCH].rearrange("p (f c) -> p f c", c=_VPITCH)
                    [:, :, 79 + 112 * si:80 + 112 * si], 1.0)

            # est strips per scale
            est = {s: pin.tile([128, meta[s][2]], bf16, tag=f"est{s}",
                               name=f"est{s}") for s in SCALES}
            # token-major attention accum, frame pitch 128 ([s2|s4|s6|junk])
            abar = pin.tile([128, F * 128], f32, tag="abar")
            abarb = pin.tile([128, F * 128], bf16, tag="abarb")
            abarT = pin.tile([128, NTOK], bf16, tag="abarT")

            # halting state
            ptn = pin.tile([1, F], f32, tag="ptn")
            Rt = pin.tile([1, F], f32, tag="Rt")
            wts = [pin.tile([1, F], f32, tag=f"w{it}", name=f"w{it}")
                   for it in range(2)]
            ssum = [[pin.tile([128, F], f32, tag=f"ssum{it}{c}",
                              name=f"ssum{it}{c}") for c in range(2)]
                    for it in range(2)]
            halt18 = pin.tile([18, 128], bf16, tag="halt18")
            nc.vector.memset(ptn[:], 0.0)
            nc.vector.memset(Rt[:], 0.0)
            nc.gpsimd.memset(abarb[:], 0.0)

            for it in range(2):
                # ============ grouped QKV projections ============
                with tc.tile_pool(name=f"pq{it}", bufs=7, space="PSUM") as ppq:
                    for gi, (wt, gt, rows) in enumerate(
                            ((wv_t, VT, 97), (wq_t, QT, 96), (wk_t, KT, 96))):
                        ptile = {}
                        for hc in range(2):
                            for h in range(2):
                                off = 0
                                for ci, w_cc in enumerate(half_cc):
                                    key = (h, ci)
                                    if hc == 0:
                                        ptile[key] = ppq.tile(
                                            [97, 512], f32, tag="pg",
                                            name="pg")
                                    nc.tensor.matmul(
                                        ptile[key][:rows, :w_cc],
                                        wt[:, hc * rows:(hc + 1) * rows],
                                        xt[hc][h][:, off:off + w_cc],
                                        start=(hc == 0), stop=(hc == 1))
                                    if hc == 1:
                                        gc = h * HALF + off
                                        eng = nc.scalar if (ci % 2 == 0)                                             else nc.vector
                                        if eng is nc.scalar:
                                            nc.scalar.copy(
                                                gt[:, gc:gc + w_cc],
                                                ptile[key][:rows, :w_cc])
                                        else:
                                            nc.vector.tensor_copy(
                                                gt[:, gc:gc + w_cc],
                                                ptile[key][:rows, :w_cc])
                                    off += w_cc

                # ============ V' build: PE transposes into vp ============
                with tc.tile_pool(name=f"pv{it}", bufs=2, space="PSUM") as ppv:
                    for t in range(F):
                        pvt = ppv.tile([128, 96], bf16, tag="pvt")
                        nc.tensor.transpose(
                            pvt[:], VT[0:96, t * 128:(t + 1) * 128],
                            identb[0:96, 0:96])
                        dst = vp[:, t * _VPITCH + 80:t * _VPITCH + 80 + 3 * 112]
                        nc.vector.tensor_copy(
                            dst.rearrange("p (s c) -> p s c", c=112)[:, :, 0:32],
                            pvt[:].rearrange("p (s c) -> p s c", c=32))

                # token-major accum cleared per iteration
                nc.gpsimd.memset(abar[:], 0.0)

                # ============ attention: strips for all scales ============
                with tc.tile_pool(name=f"ps{it}", bufs=2, space="PSUM") as pps:
                    for si, s in enumerate(SCALES):
                        offs, rng, _tot = meta[s]
                        for f2 in range(F):
                            a, b = rng[f2]
                            ncols = (b - a + 1) * 128
                            pstr = pps.tile([128, 11 * 128], f32, tag="pstr")
                            off = 0
                            for w_cc in _chunks(ncols):
                                nc.tensor.matmul(
                                    pstr[:, off:off + w_cc],
                                    KT[32 * si:32 * (si + 1),
                                       f2 * 128:(f2 + 1) * 128],
                                    QT[32 * si:32 * (si + 1),
                                       a * 128 + off:a * 128 + off + w_cc],
                                    start=True, stop=True)
                                off += w_cc
                            nc.scalar.activation(
                                est[s][:, offs[f2]:offs[f2] + ncols],
                                pstr[:, :ncols], AF.Exp)

                # ============ A@V + overlap-add per scale ============
                with (
                    tc.tile_pool(name=f"pa{it}", bufs=3, space="PSUM") as ppa,
                    tc.tile_pool(name=f"pt{it}", bufs=2, space="PSUM") as ppt,
                ):
                  for si, s in enumerate(SCALES):
                    nw = F - s + 1
                    sP = s * 128
                    offs, rng, _tot = meta[s]
                    ngrp = (nw + 2) // 3
                    gsz = {g: min(3, nw - 3 * g) for g in range(ngrp)}
                    grp = {}
                    sched = {g: [] for g in range(ngrp)}
                    for g in range(ngrp):
                        for j in range(3 * g, min(3 * g + gsz[g] + s, F)):
                            for wi in range(gsz[g]):
                                w = 3 * g + wi
                                if not (w <= j <= w + s - 1):
                                    continue
                                sched[g].append((j, wi))
                    first = {g: sched[g][0] for g in range(ngrp)}
                    last = {g: sched[g][-1] for g in range(ngrp)}

                    def drain_group(g, si=si, s=s, gsz=gsz, grp=grp):
                        sP = s * 128
                        tile = grp.pop(g)
                        av = work.tile([99, 768], bf16, tag="av_sb",
                                       bufs=3, name="av_sb")
                        eng = nc.scalar if (g % 2 == 0) else nc.vector
                        if eng is nc.scalar:
                            nc.scalar.copy(av[:, :sP], tile[:, :sP])
                        else:
                            nc.vector.tensor_copy(av[:, :sP], tile[:, :sP])
                        for qc in range(s):
                            pt = ppt.tile([128, 99], bf16, tag="ptT")
                            nc.tensor.transpose(
                                pt[:], av[:, qc * 128:(qc + 1) * 128],
                                identb[0:99, 0:99])
                            gw = gsz[g]
                            ptv = pt[:].rearrange("p (w c) -> p w c", c=33)
                            rcp = work.tile([128, 3], f32, tag="rcp")
                            nc.vector.reciprocal(rcp[:, :gw], ptv[:, :gw, 0])
                            nc.vector.tensor_tensor(
                                out=rcp[:, :gw], in0=rcp[:, :gw],
                                in1=cinv[:, si * F + 3 * g + qc:
                                         si * F + 3 * g + qc + gw],
                                op=ALU.mult)
                            resc = work.tile([128, 3 * 32], f32, tag="resc")
                            rv = resc[:].rearrange("p (w c) -> p w c", c=32)
                            nc.vector.tensor_tensor(
                                out=rv[:, :gw, :], in0=ptv[:, :gw, 1:33],
                                in1=rcp[:, :gw].broadcast_to((128, gw, 32)),
                                op=ALU.mult)
                            ab = abar[:].rearrange("p (f c) -> p f c", c=128)
                            nc.vector.tensor_tensor(
                                out=ab[:, 3 * g + qc:3 * g + qc + gw,
                                       si * 32:(si + 1) * 32],
                                in0=ab[:, 3 * g + qc:3 * g + qc + gw,
                                       si * 32:(si + 1) * 32],
                                in1=rv[:, :gw, :], op=ALU.add)

                    def frame_done(f):
                        # cast + transpose one finished abar frame
                        with nc.allow_low_precision(reason="bf16 abar, 2e-2 gate"):
                            nc.vector.tensor_copy(
                                abarb[:, f * 128:f * 128 + 96],
                                abar[:, f * 128:f * 128 + 96])
                        nc.scalar.dma_start_transpose(
                            out=abarT[:, f * 128:(f + 1) * 128],
                            in_=abarb[:].rearrange("p (f c) -> p f c", c=128)
                            [:, f, :])

                    for j in range(F):
                        for g in range(ngrp):
                            for wi in range(gsz[g]):
                                if (j, wi) not in sched[g]:
                                    continue
                                if g not in grp:
                                    grp[g] = ppa.tile([99, 768], f32,
                                                      tag="grp", name="grp")
                                w = 3 * g + wi
                                a_j = rng[j][0]
                                qoff = offs[j] + (w - a_j) * 128
                                st = (j, wi) == first[g]
                                sp = (j, wi) == last[g]
                                off = 0
                                for w_cc in _chunks(sP):
                                    outap = grp[g][0:99, off:off + w_cc]
                                    lo = (j * _VPITCH + 79 +
                                          112 * si - 33 * wi)
                                    lhs = vp[:, lo:lo + 99]
                                    nc.tensor.matmul(
                                        outap, lhs,
                                        est[s][:, qoff + off:
                                               qoff + off + w_cc],
                                        start=st, stop=sp)
                                    off += w_cc
                                if sp:
                                    drain_group(g)
                                    if si == 2:  # s=6 completes frames
                                        if g < 4:
                                            for f in range(3 * g, 3 * g + 3):
                                                frame_done(f)
                                        else:
                                            for f in range(12, 18):
                                                frame_done(f)

                # ============ halting probability ============
                nc.sync.dma_start(out=hbounce[:].rearrange("f p -> (f p)"),
                                  in_=VT[96:97, :])
                nc.sync.dma_start(out=halt18[:], in_=hbounce[:])
                with tc.tile_pool(name=f"ph{it}", bufs=1, space="PSUM") as pph:
                    elog = work.tile([18, 128], f32, tag="elog")
                    nc.scalar.activation(elog[:], halt18[:],
                                         AF.Exp, bias=nhb[:], scale=-1.0)
                    nc.vector.tensor_scalar_add(out=elog[:], in0=elog[:],
                                                scalar1=1.0)
                    sig = work.tile([18, 128], f32, tag="sig")
                    nc.vector.reciprocal(sig[:], elog[:])
                    pred = work.tile([18, 1], f32, tag="pred")
                    nc.vector.tensor_reduce(out=pred[:], in_=sig[:],
                                            axis=mybir.AxisListType.X,
                                            op=ALU.add)
                    ptp = pph.tile([1, F], f32, tag="pt")
                    nc.tensor.transpose(ptp[:], pred[:], ident[0:18, 0:18])
                    p_t = work.tile([1, F], f32, tag="p_t")
                    nc.vector.tensor_scalar_mul(out=p_t[:], in0=ptp[:],
                                                scalar1=1.0 / 128.0)

                # halting state updates (elementwise on [1,F])
                run_in = work.tile([1, F], f32, tag="run_in")
                tmp = work.tile([1, F], f32, tag="tmp")
                tmp2 = work.tile([1, F], f32, tag="tmp2")
                nh = work.tile([1, F], f32, tag="nh")
                run = work.tile([1, F], f32, tag="run")
                nc.vector.tensor_scalar(out=run_in[:], in0=ptn[:], scalar1=1.0,
                                        scalar2=None, op0=ALU.is_lt)
                nc.vector.tensor_tensor(out=tmp[:], in0=p_t[:], in1=run_in[:],
                                        op=ALU.mult)
                nc.vector.tensor_tensor(out=tmp2[:], in0=ptn[:], in1=tmp[:],
                                        op=ALU.add)
                nc.vector.tensor_scalar(out=tmp2[:], in0=tmp2[:], scalar1=0.99,
                                        scalar2=None, op0=ALU.is_gt)
                nc.vector.tensor_tensor(out=nh[:], in0=tmp2[:], in1=run_in[:],
                                        op=ALU.mult)
                nc.vector.tensor_tensor(out=run[:], in0=run_in[:], in1=nh[:],
                                        op=ALU.subtract)
                nc.vector.tensor_tensor(out=tmp[:], in0=p_t[:], in1=run[:],
                                        op=ALU.mult)
                nc.vector.tensor_tensor(out=ptn[:], in0=ptn[:], in1=tmp[:],
                                        op=ALU.add)
                nc.vector.tensor_scalar(out=tmp2[:], in0=ptn[:], scalar1=-1.0,
                                        scalar2=1.0, op0=ALU.mult, op1=ALU.add)
                nc.vector.tensor_tensor(out=tmp2[:], in0=nh[:], in1=tmp2[:],
                                        op=ALU.mult)
                nc.vector.tensor_tensor(out=Rt[:], in0=Rt[:], in1=tmp2[:],
                                        op=ALU.add)
                nc.vector.tensor_tensor(out=tmp2[:], in0=nh[:], in1=Rt[:],
                                        op=ALU.mult)
                nc.vector.tensor_tensor(out=ptn[:], in0=ptn[:], in1=tmp2[:],
                                        op=ALU.add)
                nc.vector.tensor_tensor(out=wts[it][:], in0=tmp[:], in1=tmp2[:],
                                        op=ALU.add)

                if it == 0:
                    # ---- Wo projection + bias, write xt + AllReduce ----
                    with tc.tile_pool(name="pw0", bufs=2,
                                      space="PSUM") as ppw:
                        for h in range(2):
                            for hc in range(2):
                                off = 0
                                for ci, w_cc in enumerate(half_cc):
                                    pw = ppw.tile([128, 512], f32, tag="pw")
                                    gc = h * HALF + off
                                    nc.tensor.matmul(
                                        pw[:, :w_cc],
                                        wo_t[:, hc * 128:(hc + 1) * 128],
                                        abarT[0:96, gc:gc + w_cc],
                                        start=True, stop=True)
                                    nc.vector.tensor_scalar(
                                        out=xt[hc][h][:, off:off + w_cc],
                                        in0=pw[:, :w_cc],
                                        scalar1=0.25,
                                        scalar2=boq[:, hc:hc + 1],
                                        op0=ALU.mult, op1=ALU.add)
                                    off += w_cc
                                nc.sync.dma_start(out=ar_in[h][hc],
                                                  in_=xt[hc][h][:])
                            nc.gpsimd.collective_compute(
                                "AllReduce", ALU.add,
                                ins=[ar_in[h][:]], outs=[ar_out[h][:]],
                                replica_groups=[list(range(NCORES))])
                            for hc in range(2):
                                nc.sync.dma_start(out=xt[hc][h][:],
                                                  in_=ar_out[h][hc])
                    # per-frame sums of the (post-AR) temporal
                    for hc in range(2):
                        for h in range(2):
                            nc.vector.tensor_reduce(
                                out=ssum[0][hc][:, h * 9:(h + 1) * 9],
                                in_=xt[hc][h][:].rearrange(
                                    "p (f q) -> p f q", q=128),
                                axis=mybir.AxisListType.X, op=ALU.add)
                if it == 0 and _DEBUG_DUMPS:
                    nc.sync.dma_start(out=dbg["QT"][:], in_=QT[:])
                    nc.sync.dma_start(out=dbg["VT"][:], in_=VT[:])
                    nc.sync.dma_start(out=dbg["vp"][:], in_=vp[:])
                    nc.sync.dma_start(out=dbg["est6"][:], in_=est[6][:])
                    nc.sync.dma_start(out=dbg["abar"][:], in_=abar[:])
                    nc.sync.dma_start(out=dbg["abarT"][:], in_=abarT[:])
                    nc.sync.dma_start(out=dbg["xt0"][:], in_=xt[0][:])
                    nc.sync.dma_start(out=dbg["wt0"][:], in_=wts[0][:])
                    nc.sync.dma_start(out=dbg["ssum00"][:], in_=ssum[0][0][:])
                else:
                    # ---- iter 2: only per-frame sums are needed ----
                    asum = work.tile([96, F], bf16, tag="asum", bufs=1)
                    with nc.allow_low_precision(reason="bf16 frame sums, 2e-2 gate"):
                        nc.vector.tensor_reduce(
                            out=asum[:],
                            in_=abarT[0:96, :].rearrange("p (f q) -> p f q", q=128),
                            axis=mybir.AxisListType.X, op=ALU.add)
                    with tc.tile_pool(name="pw1", bufs=1,
                                      space="PSUM") as ppw:
                        for hc in range(2):
                            ps = ppw.tile([128, F], f32, tag="ps2",
                                          name="ps2")
                            nc.tensor.matmul(
                                ps[:], wo_t[:, hc * 128:(hc + 1) * 128],
                                asum[:], start=True, stop=True)
                            nc.vector.tensor_scalar(
                                out=ssum[1][hc][:], in0=ps[:],
                                scalar1=0.25, scalar2=boq128[:, hc:hc + 1],
                                op0=ALU.mult, op1=ALU.add)

            # ============ final combine ============
            with tc.tile_pool(name="pf", bufs=1, space="PSUM") as ppf:
                w1, w2 = wts[0], wts[1]
                c1 = work.tile([1, 2 * F], f32, tag="coef")  # [c2 | c1]
                nc.vector.tensor_copy(c1[:, 0:F], w2[:])
                tmpc = work.tile([1, F], f32, tag="tmpc")
                nc.vector.tensor_scalar(out=tmpc[:], in0=w2[:], scalar1=-1.0,
                                        scalar2=1.0, op0=ALU.mult, op1=ALU.add)
                nc.vector.tensor_tensor(out=tmpc[:], in0=tmpc[:], in1=w1[:],
                                        op=ALU.mult)
                nc.vector.tensor_scalar_mul(out=c1[:, F:2 * F], in0=tmpc[:],
                                            scalar1=1.0 / NCORES)
                pc = ppf.tile([128, 2 * F], f32, tag="pc")
                nc.tensor.matmul(pc[:], ones_row[:], c1[:], start=True,
                                 stop=True)
                coefb = work.tile([128, 2 * F], f32, tag="coefb")
                nc.vector.tensor_copy(coefb[:], pc[:])
                vpart = [work.tile([128, 1], f32, tag=f"vpart{hc}",
                                   name=f"vpart{hc}") for hc in range(2)]
                for hc in range(2):
                    t2 = work.tile([128, F], f32, tag="t2")
                    nc.vector.tensor_tensor(out=t2[:], in0=ssum[1][hc][:],
                                            in1=coefb[:, 0:F], op=ALU.mult)
                    t1 = work.tile([128, F], f32, tag="t1")
                    nc.vector.tensor_tensor(out=t1[:], in0=ssum[0][hc][:],
                                            in1=coefb[:, F:2 * F], op=ALU.mult)
                    nc.vector.tensor_tensor(out=t2[:], in0=t2[:], in1=t1[:],
                                            op=ALU.add)
                    nc.vector.tensor_reduce(out=vpart[hc][:], in_=t2[:],
                                            axis=mybir.AxisListType.X,
                                            op=ALU.add)
                    nc.sync.dma_start(out=ar2_in[hc], in_=vpart[hc][:])
                nc.gpsimd.collective_compute(
                    "AllReduce", ALU.add,
                    ins=[ar2_in[:]], outs=[ar2_out[:]],
                    replica_groups=[list(range(NCORES))])
                vfull = [work.tile([128, 1], f32, tag=f"vfull{hc}",
                                   name=f"vfull{hc}") for hc in range(2)]
                ob = work.tile([1, NCLS], f32, tag="ob")
                for hc in range(2):
                    nc.sync.dma_start(out=vfull[hc][:], in_=ar2_out[hc])
                off = 0
                for w_cc in _chunks(NCLS):
                    pcls = ppf.tile([1, 512], f32, tag="pcls")
                    for hc in range(2):
                        nc.tensor.matmul(
                            pcls[:, :w_cc], vfull[hc][:],
                            clsw[:, hc * NCLS + off:hc * NCLS + off + w_cc],
                            start=(hc == 0), stop=(hc == 1))
                    nc.vector.tensor_tensor(out=ob[:, off:off + w_cc],
                                            in0=pcls[:, :w_cc],
                                            in1=clsb[:, off:off + w_cc],
                                            op=ALU.add)
                    off += w_cc
                nc.sync.dma_start(out=out_d[:], in_=ob[:])

    nc.compile()
    return nc


_NC_CACHE = None


def _get_nc():
    global _NC_CACHE
    if _NC_CACHE is None:
        _NC_CACHE = build()
    return _NC_CACHE


def _prep_in_maps(inputs):
    emb = np.ascontiguousarray(np.asarray(inputs["multiscale_embed"], np.float32))
    halt_W = np.asarray(inputs["halt_W"], np.float32)
    halt_b = np.asarray(inputs["halt_b"], np.float32)
    cls_W = np.asarray(inputs["cls_W"], np.float32)
    cls_b = np.asarray(inputs["cls_b"], np.float32)
    Wq = np.asarray(inputs["mhsa_Wq"], np.float32)
    Wk = np.asarray(inputs["mhsa_Wk"], np.float32)
    Wv = np.asarray(inputs["mhsa_Wv"], np.float32)
    Wo = np.asarray(inputs["mhsa_Wo"], np.float32)
    bo = np.asarray(inputs["mhsa_bo"], np.float32)

    import ml_dtypes
    bf = ml_dtypes.bfloat16
    xt = np.ascontiguousarray(
        emb.reshape(NTOK, H).T.reshape(2, 128, NTOK)).astype(bf)
    boq = np.ascontiguousarray(
        (0.25 * bo.sum(axis=0)).reshape(2, 128, 1)).astype(np.float32)
    hwc = halt_W.reshape(256)
    nhb = np.full((18, 1), -float(halt_b[0]), np.float32)
    cinv = np.stack([
        np.repeat((1.0 / _counts(s))[None, :], 128, axis=0) for s in SCALES
    ]).astype(np.float32)
    clsw = np.ascontiguousarray(cls_W.reshape(2, 128, NCLS))
    clsb = cls_b.reshape(1, NCLS).astype(np.float32)
    ident = np.eye(128, dtype=np.float32)
    inv_sqrt_hd = 1.0 / np.sqrt(np.float32(HD))

    in_maps = []
    for h in range(NCORES):
        sl = slice(h * HD, (h + 1) * HD)
        # grouped weights: cols = [scale2 | scale4 | scale6] head-slices
        wq_g = np.concatenate([Wq[si][:, sl] for si in range(3)], axis=1)
        wk_g = np.concatenate([Wk[si][:, sl] * inv_sqrt_hd for si in range(3)],
                              axis=1)
        wv_g = np.concatenate([Wv[si][:, sl] for si in range(3)] +
                              [hwc[:, None]], axis=1)  # [256, 97]
        wo_g = np.concatenate([Wo[si][sl, :] for si in range(3)], axis=0)
        in_maps.append({
            "xt": xt,
            "wq": np.ascontiguousarray(wq_g.reshape(2, 128, 96)).astype(bf),
            "wk": np.ascontiguousarray(wk_g.reshape(2, 128, 96)).astype(bf),
            "wv": np.ascontiguousarray(wv_g.reshape(2, 128, 97)).astype(bf),
            "wo": np.ascontiguousarray(wo_g).astype(bf),
            "boq": boq, "nhb": nhb, "cinv": cinv, "clsw": clsw, "clsb": clsb,
            "ident": ident,
        })
    return in_maps


def run(inputs, trace=False):
    _install_ntff_hook()
    from concourse.bass_utils import run_bass_kernel_spmd

    nc = _get_nc()
    in_maps = _prep_in_maps(inputs)
    res = run_bass_kernel_spmd(nc, in_maps, list(range(NCORES)), trace=trace)
    out = np.asarray(res.results[0]["out"], np.float32)
    return out, res


def kernel(**inputs):
    out, _ = run(inputs, trace=False)
    return out
